# revision 36
# baseline (speedup 1.0000x reference)
"""Trainium2 Bass kernel: transformer block with dilated (parity-strided,
banded, causal) attention.

Per-core SPMD over 8 cores: each core owns 512 tokens (+256 halo of
preceding context).  All weights are fp8(e4m3) except out_proj (bf16) and
live SBUF-resident in a host-prearranged [partition, ktile, free] layout so
every weight DMA is a long contiguous per-partition run.

Main GEMMs (QKV / FFN1 / FFN2) run in fp8 DoubleRow mode (256-deep
contraction per pass).  LayerNorm is applied directly to the fp8 GEMM
inputs: x8 = (x - mu) * rstd via two DVE passes off PSUM-broadcast rows, so
GEMM epilogues are a single scalar-engine bias/activation op.

The dilated/causal mask is applied multiplicatively to the exp() output
on the vector engine (bf16, 2x-rate), off the PE critical path.

Emission order software-pipelines the phases: attention head-pairs are
interleaved with the K/Q/V projection GEMMs, and the out-proj -> LN2 ->
FFN1 -> FFN2 chain is processed in two 256-token chunks so each serial LN
wall is covered by the other chunk's PE work.
"""

import numpy as np
import ml_dtypes

import concourse.bass as bass
import concourse.bacc as bacc
import concourse.mybir as mybir
import concourse.tile as tile
from concourse.bass_utils import run_bass_kernel_spmd

BF16NP = ml_dtypes.bfloat16
F8NP = ml_dtypes.float8_e4m3
F32 = mybir.dt.float32
BF16 = mybir.dt.bfloat16
F8 = mybir.dt.float8e4
AF = mybir.ActivationFunctionType
OP = mybir.AluOpType
DR = mybir.MatmulPerfMode.DoubleRow

P = 128
B, L, E = 2, 2048, 768
ET = E // P            # 6 tiles over E
H, D = 12, 64
MLP = 4 * E            # 3072
MT = MLP // P          # 24
OWN = 512              # tokens owned per core
HALO = 256             # preceding-context tokens
SLAB = OWN + HALO      # 768
EPS = 1e-5
N_CORES = 8


def _fold2(apv):
    """[.., T] -> [.., 2, T//2] parity view of a stride-1 token axis."""
    return apv.rearrange("... (t two) -> ... two t", two=2)


def build_program():
    nc = bacc.Bacc("TRN2", target_bir_lowering=False, debug=False)

    xT = nc.dram_tensor("xT", [E, SLAB], F32, kind="ExternalInput").ap()
    # weights pre-arranged host-side as [partition, ktile, free]
    qkv_wA = nc.dram_tensor("qkv_wA", [P, ET, 3 * E], F8, kind="ExternalInput").ap()
    out_wA = nc.dram_tensor("out_wA", [P, ET, E], BF16, kind="ExternalInput").ap()
    ffn_w1A = nc.dram_tensor("ffn_w1A", [P, ET, MLP], F8, kind="ExternalInput").ap()
    ffn_w2A = nc.dram_tensor("ffn_w2A", [P, MT, E], F8, kind="ExternalInput").ap()
    qkv_b = nc.dram_tensor("qkv_b", [3 * E], F32, kind="ExternalInput").ap()
    out_b = nc.dram_tensor("out_b", [E], F32, kind="ExternalInput").ap()
    ffn_b1 = nc.dram_tensor("ffn_b1", [MLP], F32, kind="ExternalInput").ap()
    ffn_b2 = nc.dram_tensor("ffn_b2", [E], F32, kind="ExternalInput").ap()
    maskT = nc.dram_tensor("maskT", [2, 2, P, P], BF16, kind="ExternalInput").ap()
    yT = nc.dram_tensor("yT", [E, OWN], F32, kind="ExternalOutput").ap()

    with tile.TileContext(nc) as tc:
        _emit(tc, xT, qkv_wA, out_wA, ffn_w1A, ffn_w2A,
              qkv_b, out_b, ffn_b1, ffn_b2, maskT, yT)
    nc.compile()
    return nc


def _emit(tc, xT, qkv_wA, out_wA, ffn_w1A, ffn_w2A,
          qkv_b, out_b, ffn_b1, ffn_b2, maskT, yT):
    from contextlib import ExitStack
    ctx = ExitStack()
    nc = tc.nc

    sing = ctx.enter_context(tc.tile_pool(name="sing", bufs=1))
    sq_pool = ctx.enter_context(tc.tile_pool(name="sq", bufs=2))
    ex_pool = ctx.enter_context(tc.tile_pool(name="ex", bufs=3))
    den_pool = ctx.enter_context(tc.tile_pool(name="den", bufs=3))
    row_pool = ctx.enter_context(tc.tile_pool(name="rows", bufs=4))
    rr_pool = ctx.enter_context(tc.tile_pool(name="rr", bufs=2))
    rf_pool = ctx.enter_context(tc.tile_pool(name="rf", bufs=1))
    ft_pool = ctx.enter_context(tc.tile_pool(name="ftmp", bufs=2))
    sub_pool = ctx.enter_context(tc.tile_pool(name="subt", bufs=2))

    ps_main = ctx.enter_context(tc.tile_pool(name="psg", bufs=2, space="PSUM"))
    ps_attn = ctx.enter_context(tc.tile_pool(name="pssc", bufs=3, space="PSUM"))
    ps_pv = ctx.enter_context(tc.tile_pool(name="pspv", bufs=3, space="PSUM"))

    # ---------------- phase 0: input DMAs ----------------
    x_sb = sing.tile([P, ET, SLAB], F32, tag="x_sb")
    xT_v = xT.rearrange("(o p) t -> p o t", p=P)
    for et in range(ET):
        nc.sync.dma_start(out=x_sb[:, et, :], in_=xT_v[:, et, :])

    # weights: per-partition contiguous runs, split across queues
    qkvw_sb = sing.tile([P, ET, 3 * E], F8, tag="qkvw")
    for et in range(ET):
        nc.sync.dma_start(out=qkvw_sb[:, et, :], in_=qkv_wA[:, et, :])
    outw_sb = sing.tile([P, ET, E], BF16, tag="outw")
    for et in range(ET):
        nc.sync.dma_start(out=outw_sb[:, et, :], in_=out_wA[:, et, :])
    w1_sb = sing.tile([P, ET, MLP], F8, tag="w1")
    for et in range(ET):
        nc.sync.dma_start(out=w1_sb[:, et, :], in_=ffn_w1A[:, et, :])
    w2_sb = sing.tile([P, MT, E], F8, tag="w2")
    for g in range(6):
        nc.sync.dma_start(out=w2_sb[:, 4 * g:4 * (g + 1), :],
                          in_=ffn_w2A[:, 4 * g:4 * (g + 1), :])

    qkvb_sb = sing.tile([P, 18], F32, tag="qkvb")
    nc.sync.dma_start(out=qkvb_sb, in_=qkv_b.rearrange("(o p) -> p o", p=P))
    outb_sb = sing.tile([P, ET], F32, tag="outb")
    nc.sync.dma_start(out=outb_sb, in_=out_b.rearrange("(o p) -> p o", p=P))
    b1_sb = sing.tile([P, MT], F32, tag="b1")
    nc.sync.dma_start(out=b1_sb, in_=ffn_b1.rearrange("(o p) -> p o", p=P))
    b2_sb = sing.tile([P, ET], F32, tag="b2")
    nc.sync.dma_start(out=b2_sb, in_=ffn_b2.rearrange("(o p) -> p o", p=P))

    # multiplicative score masks, replicated over the head-pair dim
    masks_sb = sing.tile([P, 2, 2, 2, P], BF16, tag="masks")
    for qb in range(2):
        for hrep in range(2):
            for kb in range(2):
                nc.sync.dma_start(out=masks_sb[:, qb, hrep, kb, :],
                                  in_=maskT[qb, kb])

    ones_pf = sing.tile([P, 1], BF16, tag="ones_pf")
    nc.vector.memset(ones_pf, 1.0)
    ones_row = sing.tile([1, P], BF16, tag="ones_row")
    nc.vector.memset(ones_row, 1.0)
    eps_sb = sing.tile([1, 1], F32, tag="eps")
    nc.vector.memset(eps_sb, EPS)

    # dummy matmuls HAM-warm the PE clock while the input DMAs stream
    warm_src = sing.tile([P, 256], BF16, tag="warm_src")
    nc.gpsimd.memset(warm_src, 0.0)
    const_bf = nc.const_aps.aps[(mybir.dt.bfloat16, 1.0)]

    def emit_warmup(n, name):
        wps = ps_attn.tile([P, 2, 2, P], F32, tag="sc", name=name)
        for wi in range(n):
            nc.tensor.matmul(wps.rearrange("p a b c -> p (a b c)")[0:1, 0:256],
                             const_bf, warm_src, start=True, stop=True)

    emit_warmup(28, "warm_ps0")

    # ---------------- layernorm -> fp8 GEMM input ----------------
    def emit_ln_stats(src, xbf_dst, ntok, x8_dst, chunks, warm=0):
        """x8_dst[:, et, :ntok] = fp8((src - mu) * rstd) over the E axis.

        xbf_dst is a bf16 scratch cast of src feeding the Sum stats matmul.
        Stats accumulate into one PSUM bank: rows (0,32) = chunk0 Sum(x),
        Sum(x^2); rows (64,96) = chunk1."""
        # per-chunk stats bank: Sum(x) @ row 0, Sum(x^2) @ row 32
        sts = [ps_attn.tile([P, 512], F32, tag="sc", name=f"st{ci}")
               for ci in range(len(chunks))]
        for et in range(ET):
            nc.vector.tensor_copy(out=xbf_dst[:, et, :ntok],
                                  in_=src[:, et, :ntok])
            xsq = sq_pool.tile([P, ntok], BF16, tag="sq")
            nc.scalar.activation(xsq, src[:, et, :ntok], AF.Square)
            for ci, (c0, cl) in enumerate(chunks):
                nc.tensor.matmul(sts[ci][0:1, :cl], ones_pf,
                                 xbf_dst[:, et, c0:c0 + cl],
                                 start=(et == 0), stop=(et == ET - 1))
                nc.tensor.matmul(sts[ci][32:33, :cl], ones_pf,
                                 xsq[:, c0:c0 + cl],
                                 start=(et == 0), stop=(et == ET - 1))
        if warm:
            emit_warmup(warm, "warm_ln")
        # per-chunk scalar chains first (frees `st` before broadcasts rotate)
        rows = []
        for ci, (c0, cl) in enumerate(chunks):
            st = sts[ci]
            mu = row_pool.tile([1, 512], F32, tag="row", name=f"mu{ci}")
            nc.scalar.activation(mu[:, :cl], st[0:1, :cl],
                                 AF.Copy, scale=1.0 / E)
            musq = row_pool.tile([1, 512], F32, tag="row", name=f"musq{ci}")
            nc.scalar.activation(musq[:, :cl], st[0:1, :cl],
                                 AF.Square, scale=1.0 / E)
            var = row_pool.tile([1, 512], F32, tag="row", name=f"var{ci}")
            nc.vector.scalar_tensor_tensor(
                out=var[:, :cl], in0=st[32:33, :cl],
                scalar=1.0 / E, in1=musq[:, :cl],
                op0=OP.mult, op1=OP.subtract)
            std = row_pool.tile([1, 512], F32, tag="row", name=f"std{ci}")
            nc.scalar.activation(std[:, :cl], var[:, :cl], AF.Sqrt, bias=eps_sb)
            af = row_pool.tile([1, 512], F32, tag="row", name=f"af{ci}")
            nc.vector.reciprocal_approx_fast(out=af[:, :cl], in_=std[:, :cl])
            a = row_pool.tile([1, 512], BF16, tag="rowb", name=f"a{ci}")
            nc.vector.tensor_copy(out=a[:, :cl], in_=af[:, :cl])
            rows.append((mu, a))
        for ci, (c0, cl) in enumerate(chunks):
            mu, a = rows[ci]
            arep = ps_attn.tile([P, 512], F32, tag="sc", name=f"arep{ci}")
            nc.tensor.matmul(arep[:, :cl], ones_row, a[:, :cl],
                             start=True, stop=True)
            mub = row_pool.tile([1, 512], BF16, tag="rowb", name=f"mub{ci}")
            nc.vector.tensor_copy(out=mub[:, :cl], in_=mu[:, :cl])
            murep = ps_attn.tile([P, 512], F32, tag="sc", name=f"murep{ci}")
            nc.tensor.matmul(murep[:, :cl], ones_row, mub[:, :cl],
                             start=True, stop=True)
            with nc.allow_low_precision(reason="fp8 GEMM inputs"):
                for et in range(ET):
                    tmp = sub_pool.tile([P, 512], BF16, tag="sub")
                    nc.vector.tensor_sub(tmp[:, :cl], src[:, et, c0:c0 + cl],
                                         murep[:, :cl])
                    nc.vector.tensor_mul(x8_dst[:, et, c0:c0 + cl],
                                         tmp[:, :cl], arep[:, :cl])

    # ---------------- phase 1: LN1 ----------------
    x1_bf = sing.tile([P, ET, SLAB], BF16, tag="x1_bf")
    x1_f8 = sing.tile([P, ET, SLAB], F8, tag="x1_f8")
    emit_ln_stats(x_sb, x1_bf, SLAB, x1_f8, [(0, 512), (512, 256)], warm=10)

    # ---------------- phase 2+3: QKV projections interleaved with attention --
    k_sb = sing.tile([P, ET, SLAB], BF16, tag="k_sb")
    q_sb = sing.tile([P, ET, OWN], BF16, tag="q_sb")
    v_sb = sing.tile([P, 2, 3, H, D + 1], BF16, tag="v_sb")
    nc.vector.memset(v_sb[:, :, :, :, D:D + 1], 1.0)
    o_sb = sing.tile([P, ET, OWN], BF16, tag="o_sb")

    def emit_k(ft):
        for c0, cl in [(0, 512), (512, 256)]:
            ps = ps_main.tile([P, 512], F32, tag="g", name=f"psk{ft}")
            for ep in range(0, ET, 2):
                nc.tensor.matmul(ps[:, :cl],
                                 qkvw_sb[:, ep:ep + 2, E + ft * P:E + (ft + 1) * P],
                                 x1_f8[:, ep:ep + 2, c0:c0 + cl],
                                 start=(ep == 0), stop=(ep == ET - 2),
                                 perf_mode=DR)
            nc.scalar.activation(k_sb[:, ft, c0:c0 + cl], ps[:, :cl],
                                 AF.Identity, bias=qkvb_sb[:, 6 + ft:7 + ft])

    def emit_q(ft):
        ps = ps_main.tile([P, 512], F32, tag="g", name=f"psq{ft}")
        for ep in range(0, ET, 2):
            nc.tensor.matmul(ps, qkvw_sb[:, ep:ep + 2, ft * P:(ft + 1) * P],
                             x1_f8[:, ep:ep + 2, HALO:SLAB],
                             start=(ep == 0), stop=(ep == ET - 2),
                             perf_mode=DR)
        nc.scalar.activation(q_sb[:, ft, :], ps, AF.Identity,
                             bias=qkvb_sb[:, ft:ft + 1])

    def emit_v(vc0, vcl):
        # V in [token, feature] orientation; V bias folded into out_b on host
        for kb in range(3):
            for par in range(2):
                ps = ps_main.tile([P, 512], F32, tag="g", name=f"psv{kb}{par}")
                for ep in range(0, ET, 2):
                    hblk = _fold2(x1_f8[:, ep:ep + 2, :])[:, :, par,
                                                          kb * P:(kb + 1) * P]
                    nc.tensor.matmul(
                        ps[:, :vcl], hblk,
                        qkvw_sb[:, ep:ep + 2, 2 * E + vc0:2 * E + vc0 + vcl],
                        start=(ep == 0), stop=(ep == ET - 2), perf_mode=DR)
                nc.scalar.activation(
                    v_sb[:, par, kb, vc0 // D:(vc0 + vcl) // D, 0:D],
                    ps[:, :vcl].rearrange("p (h d) -> p h d", d=D), AF.Copy)

    def emit_attn(h0, h1):
        kt = h0 // 2
        ro = D * (h0 % 2)
        r2 = rr_pool.tile([1, 2, OWN], BF16, tag="r2", name=f"r2_{h0}_{h1}")
        r2f = rf_pool.tile([1, 2, OWN], F32, tag="r2f", name=f"r2f_{h0}_{h1}")
        for par in range(2):
            for qb in range(2):
                sc = ps_attn.tile([P, 2, 2, P], F32, tag="sc")
                for hi, h in enumerate((h0, h1)):
                    ktt = h // 2
                    qv = _fold2(q_sb[ro:ro + D, ktt, :])[:, par,
                                                         qb * P:(qb + 1) * P]
                    kv = _fold2(k_sb[ro:ro + D, ktt, :])
                    for kbi, kb in enumerate((qb, qb + 1)):
                        nc.tensor.matmul(
                            sc[:, hi, kbi, :],
                            kv[:, par, kb * P:(kb + 1) * P], qv,
                            start=(hi == 0 and kbi == 0),
                            stop=(hi == 1 and kbi == 1))
                ex = ex_pool.tile([P, 2, 2, P], BF16, tag="ex")
                nc.scalar.activation(ex, sc, AF.Exp, scale=1.0 / np.sqrt(D))
                nc.vector.tensor_mul(ex, ex, masks_sb[:, qb])
                pv = ps_pv.tile([D + 1, 2, P], F32, tag="pv")
                for hi, h in enumerate((h0, h1)):
                    for kbi, kb in enumerate((qb, qb + 1)):
                        nc.tensor.matmul(
                            pv[:, hi, :], v_sb[:, par, kb, h, :],
                            ex[:, hi, kbi, :],
                            start=(hi == 0 and kbi == 0),
                            stop=(hi == 1 and kbi == 1))
                den = den_pool.tile([1, 2, P], F32, tag="den")
                nc.vector.tensor_copy(out=den, in_=pv[D:D + 1, :, :])
                nc.vector.reciprocal_approx_fast(
                    out=_fold2(r2f)[:, :, par, qb * P:(qb + 1) * P],
                    in_=den)
                dst = _fold2(o_sb[ro:ro + D, kt:kt + 2, :])[:, :, par,
                                                            qb * P:(qb + 1) * P]
                nc.vector.tensor_copy(out=dst, in_=pv[0:D])
        with nc.allow_low_precision(reason="bf16 softmax denom"):
            nc.vector.tensor_copy(out=r2, in_=r2f)
        for hi, tt in enumerate((kt, kt + 1)):
            rrep = ps_pv.tile([P, 512], F32, tag="pv", name=f"rr{h0}{hi}")
            nc.tensor.matmul(rrep[0:D, :], ones_row[:, 0:D], r2[:, hi, :],
                             start=True, stop=True)
            nc.vector.tensor_mul(o_sb[ro:ro + D, tt, :],
                                 o_sb[ro:ro + D, tt, :], rrep[0:D, :])

    emit_k(0); emit_k(1); emit_q(0); emit_q(1)
    emit_v(0, 512)                      # heads 0-7
    emit_attn(0, 2); emit_attn(1, 3)
    emit_k(2); emit_k(3); emit_q(2); emit_q(3)
    emit_attn(4, 6); emit_attn(5, 7)
    emit_k(4); emit_k(5); emit_q(4); emit_q(5)
    emit_v(512, 256)                    # heads 8-11
    emit_attn(8, 10); emit_attn(9, 11)
    emit_warmup(8, "warm_tail")

    # ---------------- phase 4: out-proj + residual + LN2 stats ----------------
    y1_sb = sing.tile([P, ET, OWN], F32, tag="y1_sb")
    y1_bf = sing.tile([P, ET, OWN], BF16, tag="y1_bf")
    y1_f8 = sing.tile([P, ET, OWN], F8, tag="y1_f8")
    st2 = [ps_attn.tile([P, 512], F32, tag="sc", name=f"st2c{ci}")
           for ci in range(2)]
    for et in range(ET):
        ps = ps_main.tile([P, 512], F32, tag="g", name=f"pso{et}")
        for ftl in range(ET):
            nc.tensor.matmul(ps, outw_sb[:, ftl, et * P:(et + 1) * P],
                             o_sb[:, ftl, :],
                             start=(ftl == 0), stop=(ftl == ET - 1))
        t = ft_pool.tile([P, 512], F32, tag="ft")
        nc.scalar.activation(t, ps, AF.Identity, bias=outb_sb[:, et:et + 1])
        nc.vector.tensor_add(y1_sb[:, et, :], t, x_sb[:, et, HALO:SLAB])
        # LN2 stats for this feature tile, both 256-token chunks
        nc.vector.tensor_copy(out=y1_bf[:, et, :], in_=y1_sb[:, et, :])
        ysq = sq_pool.tile([P, OWN], BF16, tag="sq")
        nc.scalar.activation(ysq, y1_sb[:, et, :], AF.Square)
        for ci in range(2):
            nc.tensor.matmul(st2[ci][0:1, :256], ones_pf,
                             y1_bf[:, et, 256 * ci:256 * (ci + 1)],
                             start=(et == 0), stop=(et == ET - 1))
            nc.tensor.matmul(st2[ci][32:33, :256], ones_pf,
                             ysq[:, 256 * ci:256 * (ci + 1)],
                             start=(et == 0), stop=(et == ET - 1))

    # LN2 epilogue rows for both chunks, then broadcast + normalize per chunk
    rows2 = []
    for ci in range(2):
        mu = row_pool.tile([1, 512], F32, tag="row", name=f"l2mu{ci}")
        nc.scalar.activation(mu[:, :256], st2[ci][0:1, :256],
                             AF.Copy, scale=1.0 / E)
        musq = row_pool.tile([1, 512], F32, tag="row", name=f"l2musq{ci}")
        nc.scalar.activation(musq[:, :256], st2[ci][0:1, :256],
                             AF.Square, scale=1.0 / E)
        var = row_pool.tile([1, 512], F32, tag="row", name=f"l2var{ci}")
        nc.vector.scalar_tensor_tensor(
            out=var[:, :256], in0=st2[ci][32:33, :256],
            scalar=1.0 / E, in1=musq[:, :256], op0=OP.mult, op1=OP.subtract)
        std = row_pool.tile([1, 512], F32, tag="row", name=f"l2std{ci}")
        nc.scalar.activation(std[:, :256], var[:, :256], AF.Sqrt, bias=eps_sb)
        af = row_pool.tile([1, 512], F32, tag="row", name=f"l2af{ci}")
        nc.vector.reciprocal_approx_fast(out=af[:, :256], in_=std[:, :256])
        a = row_pool.tile([1, 512], BF16, tag="rowb", name=f"l2a{ci}")
        nc.vector.tensor_copy(out=a[:, :256], in_=af[:, :256])
        mub = row_pool.tile([1, 512], BF16, tag="rowb", name=f"l2mub{ci}")
        nc.vector.tensor_copy(out=mub[:, :256], in_=mu[:, :256])
        rows2.append((mub, a))

    def emit_ln2_chunk(ci):
        mub, a = rows2[ci]
        c0 = 256 * ci
        arep = ps_attn.tile([P, 512], F32, tag="sc", name=f"l2arep{ci}")
        nc.tensor.matmul(arep[:, :256], ones_row, a[:, :256],
                         start=True, stop=True)
        murep = ps_attn.tile([P, 512], F32, tag="sc", name=f"l2murep{ci}")
        nc.tensor.matmul(murep[:, :256], ones_row, mub[:, :256],
                         start=True, stop=True)
        with nc.allow_low_precision(reason="fp8 GEMM inputs"):
            for et in range(ET):
                tmp = sub_pool.tile([P, 512], BF16, tag="sub")
                nc.vector.tensor_sub(tmp[:, :256], y1_sb[:, et, c0:c0 + 256],
                                     murep[:, :256])
                nc.vector.tensor_mul(y1_f8[:, et, c0:c0 + 256],
                                     tmp[:, :256], arep[:, :256])

    # ---------------- phases 5-7: FFN, 2-chunk pipelined ----------------
    ffnh = sing.tile([P, MT, OWN], F8, tag="ffnh")

    def emit_ffn1_chunk(ci):
        c0 = 256 * ci
        for mt in range(MT):
            ps = ps_main.tile([P, 512], F32, tag="g", name=f"psf{mt}c{ci}")
            for ep in range(0, ET, 2):
                nc.tensor.matmul(ps[:, :256],
                                 w1_sb[:, ep:ep + 2, mt * P:(mt + 1) * P],
                                 y1_f8[:, ep:ep + 2, c0:c0 + 256],
                                 start=(ep == 0), stop=(ep == ET - 2),
                                 perf_mode=DR)
            with nc.allow_low_precision(reason="fp8 GEMM inputs"):
                nc.scalar.activation(ffnh[:, mt, c0:c0 + 256], ps[:, :256],
                                     AF.Gelu, bias=b1_sb[:, mt:mt + 1])

    yT_view = yT.rearrange("(o p) t -> p o t", p=P)

    def emit_ffn2_chunk(ci):
        c0 = 256 * ci
        for et in range(ET):
            ps = ps_main.tile([P, 512], F32, tag="g", name=f"psd{et}c{ci}")
            for kp in range(0, MT, 2):
                nc.tensor.matmul(ps[:, :256],
                                 w2_sb[:, kp:kp + 2, et * P:(et + 1) * P],
                                 ffnh[:, kp:kp + 2, c0:c0 + 256],
                                 start=(kp == 0), stop=(kp == MT - 2),
                                 perf_mode=DR)
            t = ft_pool.tile([P, 512], F32, tag="ft")
            nc.scalar.activation(t[:, :256], ps[:, :256], AF.Identity,
                                 bias=b2_sb[:, et:et + 1])
            nc.vector.tensor_add(y1_sb[:, et, c0:c0 + 256], t[:, :256],
                                 y1_sb[:, et, c0:c0 + 256])
            nc.sync.dma_start(out=yT_view[:, et, c0:c0 + 256],
                              in_=y1_sb[:, et, c0:c0 + 256])

    emit_ln2_chunk(0)
    emit_ffn1_chunk(0)
    emit_ln2_chunk(1)
    emit_ffn1_chunk(1)
    emit_ffn2_chunk(0)
    emit_ffn2_chunk(1)

    ctx.close()


# ======================= host side =======================

def prep_inputs(x, ln1_w, ln1_b, qkv_w, qkv_b, out_w, out_b,
                ln2_w, ln2_b, ffn_w1, ffn_b1, ffn_w2, ffn_b2):
    """Shard/fold/cast the full inputs into 8 per-core input maps."""
    x = np.asarray(x, np.float32)
    f8 = lambda v: np.asarray(v, np.float64)

    def arrange(wT, kt, dtype):
        # [K, F] -> [partition, ktile, F] with K = kt*128 mapping k=(o*128+p)
        K, F = wT.shape
        return np.ascontiguousarray(
            wT.reshape(kt, P, F).transpose(1, 0, 2).astype(dtype))

    qkv_wp = f8(qkv_w) * f8(ln1_w)[None, :]
    qkv_wA = arrange(qkv_wp.T, ET, F8NP)
    qkv_b_eff = (f8(qkv_b) + f8(qkv_w) @ f8(ln1_b)).astype(np.float32)
    out_wA = arrange(f8(out_w).T, ET, BF16NP)
    out_b_eff = (f8(out_b) + f8(out_w) @ f8(qkv_b)[2 * E:]).astype(np.float32)
    ffn_w1p = f8(ffn_w1) * f8(ln2_w)[None, :]
    ffn_w1A = arrange(ffn_w1p.T, ET, F8NP)
    ffn_b1_eff = (f8(ffn_b1) + f8(ffn_w1) @ f8(ln2_b)).astype(np.float32)
    ffn_w2A = arrange(f8(ffn_w2).T, MT, F8NP)
    ffn_b2_f = np.asarray(ffn_b2, np.float32)

    cidx = np.arange(P)[:, None]   # key (folded, within block)
    ridx = np.arange(P)[None, :]   # query (folded, within block)
    m_prev = (cidx >= ridx).astype(BF16NP)
    m_diag = (cidx <= ridx).astype(BF16NP)
    zero = np.zeros((P, P), BF16NP)

    in_maps = []
    for c in range(N_CORES):
        b, ch = divmod(c, 4)
        lo = OWN * ch - HALO
        if ch == 0:
            slab = np.concatenate(
                [np.zeros((HALO, E), np.float32), x[b, 0:OWN]], axis=0)
        else:
            slab = x[b, lo:lo + SLAB]
        xTc = np.ascontiguousarray(slab.T)

        mask = np.stack([
            np.stack([zero if ch == 0 else m_prev, m_diag]),
            np.stack([m_prev, m_diag]),
        ]).astype(BF16NP)

        in_maps.append({
            "xT": xTc,
            "qkv_wA": qkv_wA, "out_wA": out_wA,
            "ffn_w1A": ffn_w1A, "ffn_w2A": ffn_w2A,
            "qkv_b": qkv_b_eff, "out_b": out_b_eff,
            "ffn_b1": ffn_b1_eff, "ffn_b2": ffn_b2_f,
            "maskT": np.ascontiguousarray(mask),
        })
    return in_maps


def gather_output(results):
    y = np.empty((B, L, E), np.float32)
    for c in range(N_CORES):
        b, ch = divmod(c, 4)
        y[b, OWN * ch:OWN * (ch + 1)] = results[c]["yT"].T
    return y


_NC_CACHE = None


def _get_program():
    global _NC_CACHE
    if _NC_CACHE is None:
        _NC_CACHE = build_program()
    return _NC_CACHE


def kernel(**inputs):
    nc = _get_program()
    in_maps = prep_inputs(**inputs)
    res = run_bass_kernel_spmd(nc, in_maps, core_ids=list(range(N_CORES)))
    return gather_output(res.results)
